# revision 28
# baseline (speedup 1.0000x reference)
"""Ragged sequence assembly on 8 TRN2 NeuronCores.

out[b] = concat([CLS, X[b, :lx[b]], RING, Xr[b, :lr[b]], END]) padded
with zeros to T = LX + LR + 3 rows of D floats.

Strategy: data-parallel over B (2 samples/core). Pure DRAM->DRAM DMA.

The host prepends CLS to each X sample and RING to each Xr sample, so
the output is two contiguous ragged segments plus END:
  seg1 = [CLS, X[b,:lx]]   -> out rows [0, 1+lx)      (src == dst offset)
  seg2 = [RING, Xr[b,:lr]] -> out rows [1+lx, 2+lx+lr) (dst = src+(1+lx))
  END                      -> out row 2+lx+lr
Each ragged segment is a branch-free binary decomposition of its length
len: one DMA per bit, blocks laid out in a fixed processing order with
offset = sum of previously processed set bits. Order is bit 0 first
(its block is then always at offset 0 with content CLS/RING, so it runs
unconditionally as a static benign overlap), then the top bit (which
only fires when len is exactly 2^top, i.e. src offset 0 - static src,
excluded from offset accumulation so skipped lower bits keep in-bounds
sources), then the rest DESCENDING so the multi-MB blocks enter the DMA
queues before the sequencer grinds through the small-bit register math.
A cleared bit adds 2^30 to the dst offset, failing the runtime bounds
check: with bounds_check="skip_entire_dma" the DMA is skipped but its
semaphore still increments, so completion counts stay static.

Engine split (the usable sequencer register pool is 49 and HWDGE/SWDGE
read offset registers asynchronously after issue, so every dynamic DMA
needs its offsets snapped into private never-overwritten registers):
SP drives sample 0, ACT drives sample 1 - seg1, seg2's big bits
(10, 9..5) and END. Pool/SWDGE (slow ~1us/DMA emission) only gets the
four tiny seg2 bits (4..1, <= 16 rows each) of both samples. Tensors
are flat int8 byte views (no stride-multiply temps) with per-sample
parameters (no sample-base temps).

The zero padding is never written: run_bass_kernel_spmd pre-zeros
ExternalOutput buffers (the PJRT path donates zeroed buffers —
bass2jax.py documents kernels rely on this).
"""

import sys

if "/opt/trn_rl_repo" not in sys.path:
    sys.path.insert(0, "/opt/trn_rl_repo")

import numpy as np

import concourse.bass as bass
import concourse.mybir as mybir
from concourse.bass_utils import run_bass_kernel_spmd

B, LX, LR, D = 16, 2048, 1024, 768
T = LX + LR + 3
RB = D * 4  # bytes per row
OOB_HUGE = 1 << 30  # far beyond any tensor extent; marks a skipped DMA
N_CORES = 8
PER_CORE = B // N_CORES  # 2

I8 = mybir.dt.int8
I32 = mybir.dt.int32

SEG2_SPLIT = 5  # seg2 bits >= SEG2_SPLIT on SP/ACT, bits 4..1 on Pool


def _emit_sample_main(eng, s, lens_sb, Xp, Xrp, END, out, sem):
    """seg1 + seg2 big bits + END for local sample s. Returns DMA count."""
    n = 0
    with (
        eng.register(f"l1_{s}") as len1_r,
        eng.register(f"l2_{s}") as len2_r,
        eng.register(f"so_{s}") as soff_r,
        eng.register(f"do_{s}") as doff_r,
        eng.register(f"mk_{s}") as msk_r,
        eng.register(f"ob_{s}") as oob_r,
    ):
        eng.reg_load([len1_r, len2_r], lens_sb[0:1, 2 * s : 2 * s + 2])

        # ---- seg1: len1 = 1+lx in [1, 2048], bits 0..11 ----
        # bit 0 unconditionally: row 0 <- Xp[0] == CLS is always correct
        eng.dma_start(out[0][0:RB], Xp[0][0:RB]).then_inc(sem, 16)
        n += 1
        eng.reg_alu(soff_r, len1_r, 1, mybir.AluOpType.bitwise_and)
        eng.reg_alu(soff_r, soff_r, RB, mybir.AluOpType.mult)
        # bit 11 fires only when len1 == 2048 -> static src, no soff
        # accumulation (keeps skipped lower bits' sources in bounds)
        eng.reg_alu(oob_r, len1_r, 1 << 11, mybir.AluOpType.bitwise_and)
        eng.reg_alu(oob_r, oob_r, 0, mybir.AluOpType.is_equal)
        eng.reg_alu(oob_r, oob_r, OOB_HUGE, mybir.AluOpType.mult)
        q = eng.snap(oob_r, donate=False, min_val=0, max_val=RB)
        eng.dma_start(
            out[0][bass.ds(q, (1 << 11) * RB)],
            Xp[0][0 : (1 << 11) * RB],
            bounds_check="skip_entire_dma",
        ).then_inc(sem, 16)
        n += 1
        # bits 10..1 descending; src == dst offset == soff
        for k in range(10, 0, -1):
            nbytes = (1 << k) * RB
            ann = ((1 << 11) + 1 - (1 << (k + 1))) * RB
            eng.reg_alu(msk_r, len1_r, 1 << k, mybir.AluOpType.bitwise_and)
            eng.reg_alu(oob_r, msk_r, 0, mybir.AluOpType.is_equal)
            eng.reg_alu(oob_r, oob_r, OOB_HUGE, mybir.AluOpType.mult)
            eng.reg_alu(oob_r, oob_r, soff_r, mybir.AluOpType.add)
            q = eng.snap(oob_r, donate=False, min_val=0, max_val=ann)
            p = eng.snap(soff_r, donate=False, min_val=0, max_val=ann)
            eng.dma_start(
                out[0][bass.ds(q, nbytes)],
                Xp[0][bass.ds(p, nbytes)],
                bounds_check="skip_entire_dma",
            ).then_inc(sem, 16)
            n += 1
            eng.reg_alu(msk_r, msk_r, RB, mybir.AluOpType.mult)
            eng.reg_alu(soff_r, soff_r, msk_r, mybir.AluOpType.add)

        # ---- seg2 big bits: len2 = 1+lr in [1, 1024], bits 0..10 ----
        eng.reg_alu(doff_r, len1_r, RB, mybir.AluOpType.mult)  # (1+lx)*RB
        # bit 0 unconditionally: row 1+lx <- Xrp[0] == RING always correct
        doff = eng.snap(doff_r, donate=False, min_val=0, max_val=(1 + LX) * RB)
        eng.dma_start(out[0][bass.ds(doff, RB)], Xrp[0][0:RB]).then_inc(sem, 16)
        n += 1
        eng.reg_alu(soff_r, len2_r, 1, mybir.AluOpType.bitwise_and)
        eng.reg_alu(soff_r, soff_r, RB, mybir.AluOpType.mult)
        # bit 10 fires only when len2 == 1024 -> static src, no accumulation
        eng.reg_alu(oob_r, len2_r, 1 << 10, mybir.AluOpType.bitwise_and)
        eng.reg_alu(oob_r, oob_r, 0, mybir.AluOpType.is_equal)
        eng.reg_alu(oob_r, oob_r, OOB_HUGE, mybir.AluOpType.mult)
        eng.reg_alu(oob_r, oob_r, doff_r, mybir.AluOpType.add)
        q = eng.snap(oob_r, donate=False, min_val=0, max_val=(1 + LX) * RB)
        eng.dma_start(
            out[0][bass.ds(q, (1 << 10) * RB)],
            Xrp[0][0 : (1 << 10) * RB],
            bounds_check="skip_entire_dma",
        ).then_inc(sem, 16)
        n += 1
        # bits 9..SEG2_SPLIT descending; dst = doff + soff
        for k in range(9, SEG2_SPLIT - 1, -1):
            nbytes = (1 << k) * RB
            anns = ((1 << 10) + 1 - (1 << (k + 1))) * RB
            annd = (1 << 11) * RB + anns
            eng.reg_alu(msk_r, len2_r, 1 << k, mybir.AluOpType.bitwise_and)
            eng.reg_alu(oob_r, msk_r, 0, mybir.AluOpType.is_equal)
            eng.reg_alu(oob_r, oob_r, OOB_HUGE, mybir.AluOpType.mult)
            eng.reg_alu(oob_r, oob_r, soff_r, mybir.AluOpType.add)
            eng.reg_alu(oob_r, oob_r, doff_r, mybir.AluOpType.add)
            q = eng.snap(oob_r, donate=False, min_val=0, max_val=annd)
            p = eng.snap(soff_r, donate=False, min_val=0, max_val=anns)
            eng.dma_start(
                out[0][bass.ds(q, nbytes)],
                Xrp[0][bass.ds(p, nbytes)],
                bounds_check="skip_entire_dma",
            ).then_inc(sem, 16)
            n += 1
            eng.reg_alu(msk_r, msk_r, RB, mybir.AluOpType.mult)
            eng.reg_alu(soff_r, soff_r, msk_r, mybir.AluOpType.add)

        # ---- END -> row 2+lx+lr == (len1 + len2) rows in ----
        eng.reg_alu(doff_r, len1_r, len2_r, mybir.AluOpType.add)
        eng.reg_alu(doff_r, doff_r, RB, mybir.AluOpType.mult)
        doff = eng.snap(doff_r, donate=False, min_val=0, max_val=(T - 1) * RB)
        eng.dma_start(out[0][bass.ds(doff, RB)], END[0][0:RB]).then_inc(sem, 16)
        n += 1
    return n


def _emit_seg2_tail(eng, s, lens_sb, Xrp, out, sem):
    """seg2 tiny bits (SEG2_SPLIT-1 .. 1) for local sample s on Pool."""
    n = 0
    with (
        eng.register(f"pl2_{s}") as len2_r,
        eng.register(f"pso_{s}") as soff_r,
        eng.register(f"pdo_{s}") as doff_r,
        eng.register(f"pmk_{s}") as msk_r,
        eng.register(f"pob_{s}") as oob_r,
    ):
        eng.reg_load([doff_r, len2_r], lens_sb[0:1, 2 * s : 2 * s + 2])
        eng.reg_alu(doff_r, doff_r, RB, mybir.AluOpType.mult)  # (1+lx)*RB
        # soff after bit0 + bits 9..SEG2_SPLIT: len2 & (1 | bits>=SPLIT mask)
        pre_mask = 1 | (((1 << 10) - 1) & ~((1 << SEG2_SPLIT) - 1))
        eng.reg_alu(soff_r, len2_r, pre_mask, mybir.AluOpType.bitwise_and)
        eng.reg_alu(soff_r, soff_r, RB, mybir.AluOpType.mult)
        for k in range(SEG2_SPLIT - 1, 0, -1):
            nbytes = (1 << k) * RB
            anns = ((1 << 10) + 1 - (1 << (k + 1))) * RB
            annd = (1 << 11) * RB + anns
            eng.reg_alu(msk_r, len2_r, 1 << k, mybir.AluOpType.bitwise_and)
            eng.reg_alu(oob_r, msk_r, 0, mybir.AluOpType.is_equal)
            eng.reg_alu(oob_r, oob_r, OOB_HUGE, mybir.AluOpType.mult)
            eng.reg_alu(oob_r, oob_r, soff_r, mybir.AluOpType.add)
            eng.reg_alu(oob_r, oob_r, doff_r, mybir.AluOpType.add)
            q = eng.snap(oob_r, donate=False, min_val=0, max_val=annd)
            p = eng.snap(soff_r, donate=False, min_val=0, max_val=anns)
            eng.dma_start(
                out[0][bass.ds(q, nbytes)],
                Xrp[0][bass.ds(p, nbytes)],
                bounds_check="skip_entire_dma",
            ).then_inc(sem, 16)
            n += 1
            eng.reg_alu(msk_r, msk_r, RB, mybir.AluOpType.mult)
            eng.reg_alu(soff_r, soff_r, msk_r, mybir.AluOpType.add)
    return n


def build_program() -> bass.Bass:
    nc = bass.Bass()

    X0 = nc.declare_dram_parameter("X0", [1, (1 + LX) * RB], I8, isOutput=False)
    X1 = nc.declare_dram_parameter("X1", [1, (1 + LX) * RB], I8, isOutput=False)
    Xr0 = nc.declare_dram_parameter("Xr0", [1, (1 + LR) * RB], I8, isOutput=False)
    Xr1 = nc.declare_dram_parameter("Xr1", [1, (1 + LR) * RB], I8, isOutput=False)
    END = nc.declare_dram_parameter("END", [1, RB], I8, isOutput=False)
    lens = nc.declare_dram_parameter("lens", [1, 2 * PER_CORE], I32, isOutput=False)
    out0 = nc.declare_dram_parameter("out0", [1, T * RB], I8, isOutput=True)
    out1 = nc.declare_dram_parameter("out1", [1, T * RB], I8, isOutput=True)

    with (
        nc.sbuf_tensor([1, 2 * PER_CORE], I32) as lens_sb,
        nc.semaphore("lens_sem") as lens_sem,
        nc.semaphore("sp_sem") as sp_sem,
        nc.semaphore("act_sem") as act_sem,
        nc.semaphore("pool_sem") as pool_sem,
        nc.Block() as block,
    ):

        @block.sync
        def _(sync):
            sync.dma_start(lens_sb[:, :], lens[:, :]).then_inc(lens_sem, 16)
            sync.wait_ge(lens_sem, 16)
            n = _emit_sample_main(sync, 0, lens_sb, X0, Xr0, END, out0, sp_sem)
            sync.wait_ge(sp_sem, n * 16)

        @block.scalar
        def _(scalar):
            scalar.wait_ge(lens_sem, 16)
            n = _emit_sample_main(scalar, 1, lens_sb, X1, Xr1, END, out1, act_sem)
            scalar.wait_ge(act_sem, n * 16)

        @block.gpsimd
        def _(gpsimd):
            gpsimd.wait_ge(lens_sem, 16)
            n = _emit_seg2_tail(gpsimd, 0, lens_sb, Xr0, out0, pool_sem)
            n += _emit_seg2_tail(gpsimd, 1, lens_sb, Xr1, out1, pool_sem)
            gpsimd.wait_ge(pool_sem, n * 16)

    _strip_entry_barrier(nc)
    return nc


def _strip_entry_barrier(nc: bass.Bass) -> None:
    """Drop the Bass-constructor preamble from the entry block: four
    const-AP memsets (unused here) and the all-engine Drain+EventSemaphore
    barrier (~7.5us on HW, Pool dge_drain included). Our own lens_sem
    ordering covers every cross-engine dependency; register setup movs
    and branches are kept."""
    blk = nc.m.functions[0].blocks[0]
    insts = blk.instructions
    kept = [
        ins
        for ins in insts
        if not (
            isinstance(ins, (mybir.InstDrain, mybir.InstEventSemaphore))
            or (isinstance(ins, mybir.InstMemset) and "const-" in str(ins))
        )
    ]
    insts[:] = kept


_NC_CACHE: list = []


def _get_nc() -> bass.Bass:
    if not _NC_CACHE:
        _NC_CACHE.append(build_program())
    return _NC_CACHE[0]


def _balance_order(lx: np.ndarray, lr: np.ndarray) -> np.ndarray:
    """Pair samples to minimize the max per-core total copy length:
    greedy largest-with-smallest, then local-search swaps."""
    tot = (lx.astype(np.int64) + lr.astype(np.int64)).ravel()
    srt = np.argsort(tot)
    pairs = [[int(srt[i]), int(srt[B - 1 - i])] for i in range(B // 2)]

    def cost():
        return sum((tot[a] + tot[b]) ** 2 for a, b in pairs)

    improved = True
    while improved:
        improved = False
        for i in range(len(pairs)):
            for j in range(i + 1, len(pairs)):
                for ii in range(2):
                    for jj in range(2):
                        old = cost()
                        pairs[i][ii], pairs[j][jj] = pairs[j][jj], pairs[i][ii]
                        if cost() < old:
                            improved = True
                        else:
                            pairs[i][ii], pairs[j][jj] = (
                                pairs[j][jj],
                                pairs[i][ii],
                            )
    order = np.empty(B, dtype=np.int64)
    for i, (a, b) in enumerate(pairs):
        order[2 * i] = a
        order[2 * i + 1] = b
    return order


def kernel(X, Xr, CLS, RING, END, lx, lr, _trace=False, _trace_kwargs=None):
    X = np.ascontiguousarray(X, dtype=np.float32)
    Xr = np.ascontiguousarray(Xr, dtype=np.float32)
    CLS = np.ascontiguousarray(CLS, dtype=np.float32).reshape(1, D)
    RING = np.ascontiguousarray(RING, dtype=np.float32).reshape(1, D)
    END = np.ascontiguousarray(END, dtype=np.float32).reshape(1, D)
    lx = np.asarray(lx, dtype=np.int32)
    lr = np.asarray(lr, dtype=np.int32)

    # [CLS; X[b]] and [RING; Xr[b]] as flat byte rows, per sample
    Xp = np.concatenate(
        [np.broadcast_to(CLS[None], (B, 1, D)), X], axis=1
    ).reshape(B, -1).view(np.int8)
    Xrp = np.concatenate(
        [np.broadcast_to(RING[None], (B, 1, D)), Xr], axis=1
    ).reshape(B, -1).view(np.int8)
    ENDb = END.reshape(1, -1).view(np.int8)

    order = _balance_order(lx, lr)

    in_maps = []
    for c in range(N_CORES):
        ids = order[c * PER_CORE : (c + 1) * PER_CORE]
        lens = np.empty((1, 2 * PER_CORE), dtype=np.int32)
        for i, b in enumerate(ids):
            lens[0, 2 * i] = 1 + lx[b]
            lens[0, 2 * i + 1] = 1 + lr[b]
        in_maps.append(
            {
                "X0": Xp[ids[0] : ids[0] + 1],
                "X1": Xp[ids[1] : ids[1] + 1],
                "Xr0": Xrp[ids[0] : ids[0] + 1],
                "Xr1": Xrp[ids[1] : ids[1] + 1],
                "END": ENDb,
                "lens": lens,
            }
        )

    nc = _get_nc()
    kres = run_bass_kernel_spmd(
        nc,
        in_maps,
        core_ids=list(range(N_CORES)),
        trace=_trace,
        **(_trace_kwargs or {}),
    )

    out = np.empty((B, T, D), dtype=np.float32)
    for c in range(N_CORES):
        ids = order[c * PER_CORE : (c + 1) * PER_CORE]
        for i, b in enumerate(ids):
            res = np.ascontiguousarray(kres.results[c][f"out{i}"]).view(np.float32)
            out[b] = res.reshape(T, D)

    if _trace:
        return out, kres
    return out
